# revision 15
# baseline (speedup 1.0000x reference)
"""Trainium2 Bass kernel for nn_Memory_66133906424236 (scatter_memory).

out[b,j] = segment_mean(comp_feats over (style,addr) keys)[key(b,j)]
           + convstack3(bias[addr(b,j)])

Strategy (8 NeuronCores, sharded over the (style,addr) key space):
  * Host computes keys from the tiny int tensors, groups the 1536 items by
    (style,addr) key, sorts groups by (addr,style) and routes whole groups
    to cores, balancing ~192 items/core. All float math happens on device.
  * Per core, the segment mean + gather-back is a matmul against a
    host-built 0/1 block-diagonal matrix A (items on partitions), with the
    conv-branch rows appended: psum = A.T @ [feats; pb], where the pb rows
    of A carry the group count k, and evacuation applies a per-row 1/k
    scale:  (sum + k*pb)/k = mean + pb.
  * The 3x conv3x3+ReLU hypernet runs on each core's addr window only
    (<=12 distinct bias rows/core) via 9 shifted sub-rectangle matmuls per
    layer (no padding buffer), then round-trips through a DRAM scratch to
    re-enter the big matmul as item-major K rows.
  * All matmuls use float32r (hw-measured 1.6e-4 rel err, 4x fp32 rate).
"""

import sys

try:
    import concourse.bass  # noqa: F401
except ImportError:
    sys.path.insert(0, "/opt/trn_rl_repo")

import numpy as np

import concourse.bacc as bacc
import concourse.mybir as mybir
import concourse.tile as tile
from concourse import bass_utils

dt = mybir.dt

# Problem constants
N_CHO, N_JUNG, N_JONG = 19, 21, 28
N_COMPS = N_CHO + N_JUNG + N_JONG  # 68
B = 512
C, H, W = 128, 16, 16
HW = H * W  # 256
F = C * HW  # 32768
N_ITEMS = B * 3  # 1536
N_CORES = 8

# Static per-core kernel shapes
M0 = 118            # out slots block 0 (K0 = these items + 10 pb rows)
M1 = 88             # out slots block 1 (K1 = these items + 10 pb rows)
NROWS = M0 + M1     # 206 feats/out rows per core
PB = 10             # conv slots == pb K rows, appended to both K blocks
K1P = M1 + PB       # 98 partitions in the K1 tile
AROWS = NROWS + PB  # 216
STRIPE = 2048       # feature-stripe (fp32 elems) per DMA/compute step
NSTRIPES = F // STRIPE  # 16
NT = STRIPE // 512  # 512-wide matmul N-tiles per stripe
NPAIR = PB // 2


def _pack(style_ids, comp_ids):
    """Group items by (style,addr) key and route groups to cores.

    Pure integer bookkeeping; returns per-core routing tables.
    """
    style = np.asarray(style_ids, dtype=np.int64)
    comp = np.asarray(comp_ids, dtype=np.int64)
    offs = np.array([0, N_CHO, N_CHO + N_JUNG], dtype=np.int64)
    addrs = (comp + offs[None, :]).reshape(-1)            # [1536]
    keys = (np.repeat(style, 3) * N_COMPS + addrs)        # [1536]

    # group items, groups ordered by (addr, style)
    order = np.lexsort((keys, addrs))
    groups = []  # (addr, item_idx_array)
    sk = keys[order]
    start = 0
    for i in range(1, len(order) + 1):
        if i == len(order) or sk[i] != sk[start]:
            items = order[start:i]
            groups.append((int(addrs[items[0]]), items))
            start = i

    # route whole groups to cores, balancing item counts
    cores = []
    gi = 0
    remaining = N_ITEMS
    for c in range(N_CORES):
        target = int(np.ceil(remaining / (N_CORES - c)))
        n = 0
        glist = []
        while gi < len(groups) and (n < target or not glist):
            k = len(groups[gi][1])
            if glist and n + k > NROWS - 8:
                break
            if glist and n >= target and n + k > target + k // 2:
                break
            glist.append(groups[gi])
            n += k
            gi += 1
        cores.append(glist)
        remaining -= n
    assert gi == len(groups), "group routing failed to place all groups"

    packed = []
    for c, glist in enumerate(cores):
        nc_items = sum(len(g[1]) for g in glist)
        assert nc_items <= NROWS, f"core {c}: {nc_items} items > {NROWS}"
        # split groups into M0/M1 at a group boundary <= M0
        n0 = 0
        split = 0
        for j, (_, items) in enumerate(glist):
            if n0 + len(items) > M0:
                break
            n0 += len(items)
            split = j + 1
        n1 = nc_items - n0
        assert n1 <= M1, f"core {c}: M1 overflow {n1} > {M1}"

        addr_lo = min(g[0] for g in glist)
        addr_hi = max(g[0] for g in glist)
        span = addr_hi - addr_lo + 1
        assert span <= PB, f"core {c}: addr span {span} > {PB}"

        amat = np.zeros((AROWS, NROWS), dtype=np.float32)
        scale = np.ones((NROWS,), dtype=np.float32)
        rows = np.zeros((NROWS,), dtype=np.int64)  # feats/out row -> item id
        used = np.zeros((NROWS,), dtype=bool)
        pos = 0
        for j, (addr, items) in enumerate(glist):
            k = len(items)
            if j == split:
                pos = M0
            sl = slice(pos, pos + k)
            rows[sl] = items
            used[sl] = True
            amat[sl, sl] = 1.0
            amat[NROWS + (addr - addr_lo), sl] = float(k)
            scale[sl] = 1.0 / k
            pos += k

        packed.append(dict(amat=amat, scale=scale, rows=rows, used=used,
                           addr_lo=addr_lo, span=span))
    return packed


_PROGRAM_CACHE = {}


def _build_program():
    """Build the (input-independent) 8-core SPMD Bass program."""
    if "p" in _PROGRAM_CACHE:
        return _PROGRAM_CACHE["p"]

    nc = bacc.Bacc("TRN2", debug=False, num_devices=N_CORES)
    feats_d = nc.dram_tensor("feats", [NROWS, F], dt.float32r,
                             kind="ExternalInput")
    amat_d = nc.dram_tensor("amat", [AROWS, NROWS], dt.float32r,
                            kind="ExternalInput")
    scale_d = nc.dram_tensor("scale", [NROWS], dt.float32,
                             kind="ExternalInput")
    biasg_d = nc.dram_tensor("biasg", [PB, C, HW], dt.float32r,
                             kind="ExternalInput")
    wt_d = nc.dram_tensor("wt", [C, 3, 3, 3, C], dt.float32r,
                          kind="ExternalInput")   # [Cin, layer, ky, kx, Cout]
    bt_d = nc.dram_tensor("bt", [C, 3], dt.float32, kind="ExternalInput")
    zer_d = nc.dram_tensor("zer", [C, PB, 18, 18], dt.float32r,
                           kind="ExternalInput")  # zeros for the pad borders
    out_d = nc.dram_tensor("outy", [NROWS, F], dt.float32,
                           kind="ExternalOutput")

    with tile.TileContext(nc) as tc:
        with tc.tile_pool(name="const", bufs=1) as cpool, \
             tc.tile_pool(name="conv", bufs=1) as vpool, \
             tc.tile_pool(name="ins", bufs=6) as inpool, \
             tc.tile_pool(name="outs", bufs=2) as outpool, \
             tc.tile_pool(name="ps", bufs=8, space="PSUM") as ps, \
             tc.tile_pool(name="dram", bufs=1, space="DRAM") as dpool:

            # ---- constants ----
            # a0: [K0=118 items + 10 pb rows, M0 cols]
            a0_t = cpool.tile([128, M0], dt.float32r, tag="a0")
            nc.sync.dma_start(out=a0_t[0:M0, :], in_=amat_d.ap()[0:M0, 0:M0])
            nc.sync.dma_start(out=a0_t[M0:128, :],
                              in_=amat_d.ap()[NROWS:AROWS, 0:M0])
            # a1: [K1=88 items + 10 pb rows, M1 cols]
            a1_t = cpool.tile([K1P, M1], dt.float32r, tag="a1")
            nc.sync.dma_start(out=a1_t[0:M1, :],
                              in_=amat_d.ap()[M0:NROWS, M0:NROWS])
            nc.sync.dma_start(out=a1_t[M1:K1P, :],
                              in_=amat_d.ap()[NROWS:AROWS, M0:NROWS])
            sc_t = cpool.tile([128, 2], dt.float32, tag="sc")
            nc.sync.dma_start(out=sc_t[0:M0, 0:1], in_=scale_d.ap()[0:M0])
            nc.sync.dma_start(out=sc_t[0:M1, 1:2], in_=scale_d.ap()[M0:NROWS])
            w_t = cpool.tile([C, 3, 3, 3, C], dt.float32r, tag="w")
            nc.sync.dma_start(out=w_t[:, :, :, :, :], in_=wt_d.ap())
            b_t = cpool.tile([C, 3], dt.float32, tag="b")
            nc.sync.dma_start(out=b_t[:, :], in_=bt_d.ap())

            # ---- conv hypernet on this core's addr window ----
            pb_dram = dpool.tile([PB, F], dt.float32r, tag="pbd")
            cur = vpool.tile([C, PB, H, W], dt.float32r, tag="cur")
            nxt = vpool.tile([C, PB, H, W], dt.float32r, tag="nxt")
            pad = vpool.tile([C, PB, 18, 18], dt.float32r, tag="pad")
            nc.gpsimd.dma_start(out=pad[:, :, :, :], in_=zer_d.ap())
            nc.gpsimd.dma_start(
                out=cur[:, :, :, :].rearrange("c a h w -> c a (h w)"),
                in_=biasg_d.ap().rearrange("a c f -> c a f"))
            for l in range(3):
                src, dst = (cur, nxt) if l % 2 == 0 else (nxt, cur)
                # interior refresh; borders stay zero from the one-time init
                nc.vector.tensor_copy(pad[:, :, 1:17, 1:17],
                                      src[:, :, :, :])
                for p in range(NPAIR):
                    pcv = ps.tile([128, 2, H, W], dt.float32, tag="ps")
                    for si in range(9):
                        ky, kx = divmod(si, 3)
                        nc.tensor.matmul(
                            pcv[:, :, :, :],
                            w_t[:, l, ky, kx, :],
                            pad[:, 2 * p:2 * p + 2, ky:ky + H, kx:kx + W],
                            start=(si == 0), stop=(si == 8))
                    nc.scalar.activation(
                        dst[:, 2 * p:2 * p + 2, :, :], pcv[:, :, :, :],
                        mybir.ActivationFunctionType.Relu,
                        bias=b_t[:, l:l + 1], scale=1.0)
            res = nxt  # after 3 layers
            nc.gpsimd.dma_start(
                out=pb_dram[:, :].rearrange("a (c f) -> c a f", c=C),
                in_=res[:, :, :, :].rearrange("c a h w -> c a (h w)"))

            # ---- streamed segment-mean + pb matmul over feature stripes ----
            for s in range(NSTRIPES):
                fs = slice(s * STRIPE, (s + 1) * STRIPE)
                k0 = inpool.tile([128, STRIPE], dt.float32r, tag="k0")
                nc.sync.dma_start(out=k0[0:M0, :], in_=feats_d.ap()[0:M0, fs])
                nc.gpsimd.dma_start(out=k0[M0:128, :], in_=pb_dram[:, fs])
                k1 = inpool.tile([K1P, STRIPE], dt.float32r, tag="k1")
                nc.sync.dma_start(out=k1[0:M1, :],
                                  in_=feats_d.ap()[M0:NROWS, fs])
                nc.gpsimd.dma_start(out=k1[M1:K1P, :], in_=pb_dram[:, fs])
                o0 = outpool.tile([M0, STRIPE], dt.float32, tag="o0")
                o1 = outpool.tile([M1, STRIPE], dt.float32, tag="o1")
                for t in range(NT):
                    ts = slice(t * 512, (t + 1) * 512)
                    p0 = ps.tile([128, 512], dt.float32, tag="ps")
                    nc.tensor.matmul(p0[0:M0, :], a0_t[:, :], k0[:, ts],
                                     start=True, stop=True)
                    p1 = ps.tile([128, 512], dt.float32, tag="ps")
                    nc.tensor.matmul(p1[0:M1, :], a1_t[:, :], k1[:, ts],
                                     start=True, stop=True)
                    nc.vector.tensor_scalar_mul(o0[:, ts], p0[0:M0, :],
                                                sc_t[0:M0, 0:1])
                    nc.scalar.mul(o1[:, ts], p1[0:M1, :],
                                  sc_t[0:M1, 1:2])
                nc.scalar.dma_start(out=out_d.ap()[0:M0, fs], in_=o0[:, :])
                nc.scalar.dma_start(out=out_d.ap()[M0:NROWS, fs],
                                    in_=o1[:, :])

    nc.compile()
    _PROGRAM_CACHE["p"] = nc
    return nc


def _run(inputs, trace=False, trace_cores=None):
    style_ids = np.asarray(inputs["style_ids"])
    comp_ids = np.asarray(inputs["comp_ids"])
    comp_feats = np.ascontiguousarray(
        np.asarray(inputs["comp_feats"], dtype=np.float32))
    bias = np.asarray(inputs["bias"], dtype=np.float32).reshape(N_COMPS, C, HW)
    ws = [np.asarray(inputs[k], dtype=np.float32) for k in ("w1", "w2", "w3")]
    bs = [np.asarray(inputs[k], dtype=np.float32) for k in ("b1", "b2", "b3")]

    packed = _pack(style_ids, comp_ids)
    feats_flat = comp_feats.reshape(N_ITEMS, F)

    # [Cin, layer, ky, kx, Cout] from three [Cout, Cin, ky, kx]
    wt = np.transpose(np.stack(ws, axis=0), (2, 0, 3, 4, 1))
    wt = np.ascontiguousarray(wt, dtype=np.float32)
    bt = np.ascontiguousarray(np.stack(bs, axis=0).T, dtype=np.float32)

    in_maps = []
    for pk in packed:
        feats_c = np.zeros((NROWS, F), dtype=np.float32)
        feats_c[pk["used"]] = feats_flat[pk["rows"][pk["used"]]]
        biasg = np.zeros((PB, C, HW), dtype=np.float32)
        nsl = min(PB, N_COMPS - pk["addr_lo"])
        biasg[:nsl] = bias[pk["addr_lo"]:pk["addr_lo"] + nsl]
        in_maps.append({
            "feats": feats_c,
            "amat": pk["amat"],
            "scale": pk["scale"],
            "biasg": biasg,
            "wt": wt,
            "bt": bt,
            "zer": np.zeros((C, PB, 18, 18), dtype=np.float32),
        })

    nc = _build_program()
    res = bass_utils.run_bass_kernel_spmd(
        nc, in_maps, core_ids=list(range(N_CORES)), trace=trace,
        trace_cores=trace_cores)

    out_flat = np.zeros((N_ITEMS, F), dtype=np.float32)
    for pk, om in zip(packed, res.results):
        oc = om["outy"]
        out_flat[pk["rows"][pk["used"]]] = oc[pk["used"]]
    out = out_flat.reshape(B, 3, C, H, W)
    return out, res


def kernel(**inputs):
    out, _ = _run(inputs, trace=False)
    return out


# revision 17
# speedup vs baseline: 1.0157x; 1.0157x over previous
"""Trainium2 Bass kernel for nn_Memory_66133906424236 (scatter_memory).

out[b,j] = segment_mean(comp_feats over (style,addr) keys)[key(b,j)]
           + convstack3(bias[addr(b,j)])

Strategy (8 NeuronCores, sharded over the (style,addr) key space):
  * Host computes keys from the tiny int tensors, groups the 1536 items by
    (style,addr) key, sorts groups by (addr,style) and routes whole groups
    to cores, balancing ~192 items/core. All float math happens on device.
  * Per core, the segment mean + gather-back is a matmul against a
    host-built 0/1 block-diagonal matrix A (items on partitions), with the
    conv-branch rows appended: psum = A.T @ [feats; pb], where the pb rows
    of A carry the group count k, and evacuation applies a per-row 1/k
    scale:  (sum + k*pb)/k = mean + pb.
  * The 3x conv3x3+ReLU hypernet runs on each core's addr window only
    (<=12 distinct bias rows/core) via 9 shifted sub-rectangle matmuls per
    layer (no padding buffer), then round-trips through a DRAM scratch to
    re-enter the big matmul as item-major K rows.
  * All matmuls use float32r (hw-measured 1.6e-4 rel err, 4x fp32 rate).
"""

import sys

try:
    import concourse.bass  # noqa: F401
except ImportError:
    sys.path.insert(0, "/opt/trn_rl_repo")

import numpy as np

import concourse.bacc as bacc
import concourse.mybir as mybir
import concourse.tile as tile
from concourse import bass_utils

dt = mybir.dt

# Problem constants
N_CHO, N_JUNG, N_JONG = 19, 21, 28
N_COMPS = N_CHO + N_JUNG + N_JONG  # 68
B = 512
C, H, W = 128, 16, 16
HW = H * W  # 256
F = C * HW  # 32768
N_ITEMS = B * 3  # 1536
N_CORES = 8

# Static per-core kernel shapes
M0 = 118            # out slots block 0 (K0 = these items + 10 pb rows)
M1 = 88             # out slots block 1 (K1 = these items + 10 pb rows)
NROWS = M0 + M1     # 206 feats/out rows per core
PB = 10             # conv slots == pb K rows, appended to both K blocks
K1P = M1 + PB       # 98 partitions in the K1 tile
AROWS = NROWS + PB  # 216
STRIPE = 2048       # feature-stripe (fp32 elems) per DMA/compute step
NSTRIPES = F // STRIPE  # 16
NT = STRIPE // 512  # 512-wide matmul N-tiles per stripe
NPAIR = PB // 2


def _pack(style_ids, comp_ids):
    """Group items by (style,addr) key and route groups to cores.

    Pure integer bookkeeping; returns per-core routing tables.
    """
    style = np.asarray(style_ids, dtype=np.int64)
    comp = np.asarray(comp_ids, dtype=np.int64)
    offs = np.array([0, N_CHO, N_CHO + N_JUNG], dtype=np.int64)
    addrs = (comp + offs[None, :]).reshape(-1)            # [1536]
    keys = (np.repeat(style, 3) * N_COMPS + addrs)        # [1536]

    # group items, groups ordered by (addr, style)
    order = np.lexsort((keys, addrs))
    groups = []  # (addr, item_idx_array)
    sk = keys[order]
    start = 0
    for i in range(1, len(order) + 1):
        if i == len(order) or sk[i] != sk[start]:
            items = order[start:i]
            groups.append((int(addrs[items[0]]), items))
            start = i

    # route whole groups to cores, balancing item counts
    cores = []
    gi = 0
    remaining = N_ITEMS
    for c in range(N_CORES):
        target = int(np.ceil(remaining / (N_CORES - c)))
        n = 0
        glist = []
        while gi < len(groups) and (n < target or not glist):
            k = len(groups[gi][1])
            if glist and n + k > NROWS - 8:
                break
            if glist and n >= target and n + k > target + k // 2:
                break
            glist.append(groups[gi])
            n += k
            gi += 1
        cores.append(glist)
        remaining -= n
    assert gi == len(groups), "group routing failed to place all groups"

    packed = []
    for c, glist in enumerate(cores):
        nc_items = sum(len(g[1]) for g in glist)
        assert nc_items <= NROWS, f"core {c}: {nc_items} items > {NROWS}"
        # split groups into M0/M1 at a group boundary <= M0
        n0 = 0
        split = 0
        for j, (_, items) in enumerate(glist):
            if n0 + len(items) > M0:
                break
            n0 += len(items)
            split = j + 1
        n1 = nc_items - n0
        assert n1 <= M1, f"core {c}: M1 overflow {n1} > {M1}"

        addr_lo = min(g[0] for g in glist)
        addr_hi = max(g[0] for g in glist)
        span = addr_hi - addr_lo + 1
        assert span <= PB, f"core {c}: addr span {span} > {PB}"

        amat = np.zeros((AROWS, NROWS), dtype=np.float32)
        scale = np.ones((NROWS,), dtype=np.float32)
        rows = np.zeros((NROWS,), dtype=np.int64)  # feats/out row -> item id
        used = np.zeros((NROWS,), dtype=bool)
        pos = 0
        for j, (addr, items) in enumerate(glist):
            k = len(items)
            if j == split:
                pos = M0
            sl = slice(pos, pos + k)
            rows[sl] = items
            used[sl] = True
            amat[sl, sl] = 1.0
            amat[NROWS + (addr - addr_lo), sl] = float(k)
            scale[sl] = 1.0 / k
            pos += k

        packed.append(dict(amat=amat, scale=scale, rows=rows, used=used,
                           addr_lo=addr_lo, span=span))
    return packed


_PROGRAM_CACHE = {}


def _build_program():
    """Build the (input-independent) 8-core SPMD Bass program."""
    if "p" in _PROGRAM_CACHE:
        return _PROGRAM_CACHE["p"]

    nc = bacc.Bacc("TRN2", debug=False, num_devices=N_CORES)
    feats_d = nc.dram_tensor("feats", [NROWS, F], dt.float32r,
                             kind="ExternalInput")
    amat_d = nc.dram_tensor("amat", [AROWS, NROWS], dt.float32r,
                            kind="ExternalInput")
    scale_d = nc.dram_tensor("scale", [NROWS], dt.float32,
                             kind="ExternalInput")
    biasg_d = nc.dram_tensor("biasg", [PB, C, HW], dt.float32r,
                             kind="ExternalInput")
    wt_d = nc.dram_tensor("wt", [C, 3, 3, 3, C], dt.float32r,
                          kind="ExternalInput")   # [Cin, layer, ky, kx, Cout]
    bt_d = nc.dram_tensor("bt", [C, 3], dt.float32, kind="ExternalInput")
    zer_d = nc.dram_tensor("zer", [C, PB, 18, 18], dt.float32r,
                           kind="ExternalInput")  # zeros for the pad borders
    out_d = nc.dram_tensor("outy", [NROWS, F], dt.float32,
                           kind="ExternalOutput")

    with tile.TileContext(nc) as tc:
        with tc.tile_pool(name="const", bufs=1) as cpool, \
             tc.tile_pool(name="conv", bufs=1) as vpool, \
             tc.tile_pool(name="ins", bufs=6) as inpool, \
             tc.tile_pool(name="outs", bufs=2) as outpool, \
             tc.tile_pool(name="ps", bufs=8, space="PSUM") as ps, \
             tc.tile_pool(name="dram", bufs=1, space="DRAM") as dpool:

            # ---- constants ----
            # a0: [K0=118 items + 10 pb rows, M0 cols]
            a0_t = cpool.tile([128, M0], dt.float32r, tag="a0")
            nc.sync.dma_start(out=a0_t[0:M0, :], in_=amat_d.ap()[0:M0, 0:M0])
            nc.sync.dma_start(out=a0_t[M0:128, :],
                              in_=amat_d.ap()[NROWS:AROWS, 0:M0])
            # a1: [K1=88 items + 10 pb rows, M1 cols]
            a1_t = cpool.tile([K1P, M1], dt.float32r, tag="a1")
            nc.sync.dma_start(out=a1_t[0:M1, :],
                              in_=amat_d.ap()[M0:NROWS, M0:NROWS])
            nc.sync.dma_start(out=a1_t[M1:K1P, :],
                              in_=amat_d.ap()[NROWS:AROWS, M0:NROWS])
            sc_t = cpool.tile([128, 2], dt.float32, tag="sc")
            nc.sync.dma_start(out=sc_t[0:M0, 0:1], in_=scale_d.ap()[0:M0])
            nc.sync.dma_start(out=sc_t[0:M1, 1:2], in_=scale_d.ap()[M0:NROWS])
            w_t = cpool.tile([C, 3, 3, 3, C], dt.float32r, tag="w")
            nc.sync.dma_start(out=w_t[:, :, :, :, :], in_=wt_d.ap())
            b_t = cpool.tile([C, 3], dt.float32, tag="b")
            nc.sync.dma_start(out=b_t[:, :], in_=bt_d.ap())

            # ---- prefetch the first stripes' feats while the conv runs ----
            PREF = 6
            ktiles = {}
            for s in range(PREF):
                fs = slice(s * STRIPE, (s + 1) * STRIPE)
                k0 = inpool.tile([128, STRIPE], dt.float32r, tag="k0")
                nc.sync.dma_start(out=k0[0:M0, :], in_=feats_d.ap()[0:M0, fs])
                k1 = inpool.tile([K1P, STRIPE], dt.float32r, tag="k1")
                nc.sync.dma_start(out=k1[0:M1, :],
                                  in_=feats_d.ap()[M0:NROWS, fs])
                ktiles[s] = (k0, k1)

            # ---- conv hypernet on this core's addr window ----
            pb_dram = dpool.tile([PB, F], dt.float32r, tag="pbd")
            cur = vpool.tile([C, PB, H, W], dt.float32r, tag="cur")
            nxt = vpool.tile([C, PB, H, W], dt.float32r, tag="nxt")
            pad = vpool.tile([C, PB, 18, 18], dt.float32r, tag="pad")
            nc.gpsimd.dma_start(out=pad[:, :, :, :], in_=zer_d.ap())
            nc.gpsimd.dma_start(
                out=cur[:, :, :, :].rearrange("c a h w -> c a (h w)"),
                in_=biasg_d.ap().rearrange("a c f -> c a f"))
            for l in range(3):
                src, dst = (cur, nxt) if l % 2 == 0 else (nxt, cur)
                # interior refresh; borders stay zero from the one-time init
                nc.vector.tensor_copy(pad[:, :, 1:17, 1:17],
                                      src[:, :, :, :])
                for p in range(NPAIR):
                    pcv = ps.tile([128, 2, H, W], dt.float32, tag="ps")
                    for si in range(9):
                        ky, kx = divmod(si, 3)
                        nc.tensor.matmul(
                            pcv[:, :, :, :],
                            w_t[:, l, ky, kx, :],
                            pad[:, 2 * p:2 * p + 2, ky:ky + H, kx:kx + W],
                            start=(si == 0), stop=(si == 8))
                    nc.scalar.activation(
                        dst[:, 2 * p:2 * p + 2, :, :], pcv[:, :, :, :],
                        mybir.ActivationFunctionType.Relu,
                        bias=b_t[:, l:l + 1], scale=1.0)
            res = nxt  # after 3 layers
            nc.gpsimd.dma_start(
                out=pb_dram[:, :].rearrange("a (c f) -> c a f", c=C),
                in_=res[:, :, :, :].rearrange("c a h w -> c a (h w)"))

            # ---- streamed segment-mean + pb matmul over feature stripes ----
            for s in range(NSTRIPES):
                fs = slice(s * STRIPE, (s + 1) * STRIPE)
                if s < PREF:
                    k0, k1 = ktiles.pop(s)
                else:
                    k0 = inpool.tile([128, STRIPE], dt.float32r, tag="k0")
                    nc.sync.dma_start(out=k0[0:M0, :],
                                      in_=feats_d.ap()[0:M0, fs])
                    k1 = inpool.tile([K1P, STRIPE], dt.float32r, tag="k1")
                    nc.sync.dma_start(out=k1[0:M1, :],
                                      in_=feats_d.ap()[M0:NROWS, fs])
                nc.sync.dma_start(out=k0[M0:128, :], in_=pb_dram[:, fs])
                nc.sync.dma_start(out=k1[M1:K1P, :], in_=pb_dram[:, fs])
                o0 = outpool.tile([M0, STRIPE], dt.float32, tag="o0")
                o1 = outpool.tile([M1, STRIPE], dt.float32, tag="o1")
                for t in range(NT):
                    ts = slice(t * 512, (t + 1) * 512)
                    p0 = ps.tile([128, 512], dt.float32, tag="ps")
                    nc.tensor.matmul(p0[0:M0, :], a0_t[:, :], k0[:, ts],
                                     start=True, stop=True)
                    p1 = ps.tile([128, 512], dt.float32, tag="ps")
                    nc.tensor.matmul(p1[0:M1, :], a1_t[:, :], k1[:, ts],
                                     start=True, stop=True)
                    nc.vector.tensor_scalar_mul(o0[:, ts], p0[0:M0, :],
                                                sc_t[0:M0, 0:1])
                    nc.scalar.mul(o1[:, ts], p1[0:M1, :],
                                  sc_t[0:M1, 1:2])
                nc.scalar.dma_start(out=out_d.ap()[0:M0, fs], in_=o0[:, :])
                nc.scalar.dma_start(out=out_d.ap()[M0:NROWS, fs],
                                    in_=o1[:, :])

    nc.compile()
    _PROGRAM_CACHE["p"] = nc
    return nc


def _run(inputs, trace=False, trace_cores=None):
    style_ids = np.asarray(inputs["style_ids"])
    comp_ids = np.asarray(inputs["comp_ids"])
    comp_feats = np.ascontiguousarray(
        np.asarray(inputs["comp_feats"], dtype=np.float32))
    bias = np.asarray(inputs["bias"], dtype=np.float32).reshape(N_COMPS, C, HW)
    ws = [np.asarray(inputs[k], dtype=np.float32) for k in ("w1", "w2", "w3")]
    bs = [np.asarray(inputs[k], dtype=np.float32) for k in ("b1", "b2", "b3")]

    packed = _pack(style_ids, comp_ids)
    feats_flat = comp_feats.reshape(N_ITEMS, F)

    # [Cin, layer, ky, kx, Cout] from three [Cout, Cin, ky, kx]
    wt = np.transpose(np.stack(ws, axis=0), (2, 0, 3, 4, 1))
    wt = np.ascontiguousarray(wt, dtype=np.float32)
    bt = np.ascontiguousarray(np.stack(bs, axis=0).T, dtype=np.float32)

    in_maps = []
    for pk in packed:
        feats_c = np.zeros((NROWS, F), dtype=np.float32)
        feats_c[pk["used"]] = feats_flat[pk["rows"][pk["used"]]]
        biasg = np.zeros((PB, C, HW), dtype=np.float32)
        nsl = min(PB, N_COMPS - pk["addr_lo"])
        biasg[:nsl] = bias[pk["addr_lo"]:pk["addr_lo"] + nsl]
        in_maps.append({
            "feats": feats_c,
            "amat": pk["amat"],
            "scale": pk["scale"],
            "biasg": biasg,
            "wt": wt,
            "bt": bt,
            "zer": np.zeros((C, PB, 18, 18), dtype=np.float32),
        })

    nc = _build_program()
    res = bass_utils.run_bass_kernel_spmd(
        nc, in_maps, core_ids=list(range(N_CORES)), trace=trace,
        trace_cores=trace_cores)

    out_flat = np.zeros((N_ITEMS, F), dtype=np.float32)
    for pk, om in zip(packed, res.results):
        oc = om["outy"]
        out_flat[pk["rows"][pk["used"]]] = oc[pk["used"]]
    out = out_flat.reshape(B, 3, C, H, W)
    return out, res


def kernel(**inputs):
    out, _ = _run(inputs, trace=False)
    return out


# revision 19
# speedup vs baseline: 2.5497x; 2.5103x over previous
"""Trainium2 Bass kernel for nn_Memory_66133906424236 (scatter_memory).

out[b,j] = segment_mean(comp_feats over (style,addr) keys)[key(b,j)]
           + convstack3(bias[addr(b,j)])

Strategy (8 NeuronCores, sharded over the (style,addr) key space):
  * Host computes keys from the tiny int tensors, groups the 1536 items by
    (style,addr) key, sorts groups by (addr,style) and routes whole groups
    to cores, balancing ~192 items/core. All float math happens on device.
  * Per core, the segment mean + gather-back is a matmul against a
    host-built 0/1 block-diagonal matrix A (items on partitions), with the
    conv-branch rows appended: psum = A.T @ [feats; pb], where the pb rows
    of A carry the group count k, and evacuation applies a per-row 1/k
    scale:  (sum + k*pb)/k = mean + pb.
  * The 3x conv3x3+ReLU hypernet runs on each core's addr window only
    (<=12 distinct bias rows/core) via 9 shifted sub-rectangle matmuls per
    layer (no padding buffer), then round-trips through a DRAM scratch to
    re-enter the big matmul as item-major K rows.
  * All matmuls use float32r (hw-measured 1.6e-4 rel err, 4x fp32 rate).
"""

import sys

try:
    import concourse.bass  # noqa: F401
except ImportError:
    sys.path.insert(0, "/opt/trn_rl_repo")

import numpy as np

import concourse.bacc as bacc
import concourse.mybir as mybir
import concourse.tile as tile
from concourse import bass_utils

dt = mybir.dt

# Problem constants
N_CHO, N_JUNG, N_JONG = 19, 21, 28
N_COMPS = N_CHO + N_JUNG + N_JONG  # 68
B = 512
C, H, W = 128, 16, 16
HW = H * W  # 256
F = C * HW  # 32768
N_ITEMS = B * 3  # 1536
N_CORES = 8

# Static per-core kernel shapes. All bulk DMAs must be exactly 128
# partitions (hw-measured: non-128-partition transfers run ~7x slower),
# so rows are laid out as two dense 128-row blocks with zero padding.
M0 = 128            # item capacity of row block 0 (dram rows 0..127)
M1 = 88             # item capacity of row block 1 (dram rows 128..215)
ROWS_D = 256        # dram rows per core (2 x 128; rows beyond items are 0)
PB = 10             # conv slots == pb K rows (own small tile + K=10 matmuls)
AROWS = ROWS_D + PB  # 266 rows in the A matrix
STRIPE = 2048       # feature-stripe (fp32 elems) per DMA/compute step
NSTRIPES = F // STRIPE  # 16
NT = STRIPE // 512  # 512-wide matmul N-tiles per stripe
NPAIR = PB // 2


def _pack(style_ids, comp_ids):
    """Group items by (style,addr) key and route groups to cores.

    Pure integer bookkeeping; returns per-core routing tables.
    """
    style = np.asarray(style_ids, dtype=np.int64)
    comp = np.asarray(comp_ids, dtype=np.int64)
    offs = np.array([0, N_CHO, N_CHO + N_JUNG], dtype=np.int64)
    addrs = (comp + offs[None, :]).reshape(-1)            # [1536]
    keys = (np.repeat(style, 3) * N_COMPS + addrs)        # [1536]

    # group items, groups ordered by (addr, style)
    order = np.lexsort((keys, addrs))
    groups = []  # (addr, item_idx_array)
    sk = keys[order]
    start = 0
    for i in range(1, len(order) + 1):
        if i == len(order) or sk[i] != sk[start]:
            items = order[start:i]
            groups.append((int(addrs[items[0]]), items))
            start = i

    # route whole groups to cores, balancing item counts
    cores = []
    gi = 0
    remaining = N_ITEMS
    for c in range(N_CORES):
        target = int(np.ceil(remaining / (N_CORES - c)))
        n = 0
        glist = []
        while gi < len(groups) and (n < target or not glist):
            k = len(groups[gi][1])
            if glist and n + k > M0 + M1 - 8:
                break
            if glist and n >= target and n + k > target + k // 2:
                break
            glist.append(groups[gi])
            n += k
            gi += 1
        cores.append(glist)
        remaining -= n
    assert gi == len(groups), "group routing failed to place all groups"

    packed = []
    for c, glist in enumerate(cores):
        nc_items = sum(len(g[1]) for g in glist)
        assert nc_items <= M0 + M1, f"core {c}: {nc_items} items"
        # split groups into M0/M1 at a group boundary <= M0
        n0 = 0
        split = 0
        for j, (_, items) in enumerate(glist):
            if n0 + len(items) > M0:
                break
            n0 += len(items)
            split = j + 1
        n1 = nc_items - n0
        assert n1 <= M1, f"core {c}: M1 overflow {n1} > {M1}"

        addr_lo = min(g[0] for g in glist)
        addr_hi = max(g[0] for g in glist)
        span = addr_hi - addr_lo + 1
        assert span <= PB, f"core {c}: addr span {span} > {PB}"

        amat = np.zeros((AROWS, ROWS_D), dtype=np.float32)
        scale = np.ones((ROWS_D,), dtype=np.float32)
        rows = np.zeros((ROWS_D,), dtype=np.int64)  # feats/out row -> item id
        used = np.zeros((ROWS_D,), dtype=bool)
        pos = 0
        for j, (addr, items) in enumerate(glist):
            k = len(items)
            if j == split:
                pos = 128
            sl = slice(pos, pos + k)
            rows[sl] = items
            used[sl] = True
            amat[sl, sl] = 1.0
            amat[ROWS_D + (addr - addr_lo), sl] = float(k)
            scale[sl] = 1.0 / k
            pos += k

        packed.append(dict(amat=amat, scale=scale, rows=rows, used=used,
                           addr_lo=addr_lo, span=span))
    return packed


_PROGRAM_CACHE = {}


def _build_program():
    """Build the (input-independent) 8-core SPMD Bass program."""
    if "p" in _PROGRAM_CACHE:
        return _PROGRAM_CACHE["p"]

    nc = bacc.Bacc("TRN2", debug=False, num_devices=N_CORES)
    feats_d = nc.dram_tensor("feats", [ROWS_D, F], dt.float32r,
                             kind="ExternalInput")
    amat_d = nc.dram_tensor("amat", [AROWS, ROWS_D], dt.float32r,
                            kind="ExternalInput")
    scale_d = nc.dram_tensor("scale", [ROWS_D], dt.float32,
                             kind="ExternalInput")
    biasg_d = nc.dram_tensor("biasg", [PB, C, HW], dt.float32r,
                             kind="ExternalInput")
    wt_d = nc.dram_tensor("wt", [C, 3, 3, 3, C], dt.float32r,
                          kind="ExternalInput")   # [Cin, layer, ky, kx, Cout]
    bt_d = nc.dram_tensor("bt", [C, 3], dt.float32, kind="ExternalInput")
    zer_d = nc.dram_tensor("zer", [C, PB, 18, 18], dt.float32r,
                           kind="ExternalInput")  # zeros for the pad borders
    out_d = nc.dram_tensor("outy", [ROWS_D, F], dt.float32,
                           kind="ExternalOutput")

    with tile.TileContext(nc) as tc:
        with tc.tile_pool(name="const", bufs=1) as cpool, \
             tc.tile_pool(name="conv", bufs=1) as vpool, \
             tc.tile_pool(name="ins", bufs=5) as inpool, \
             tc.tile_pool(name="pbp", bufs=3) as pbpool, \
             tc.tile_pool(name="outs", bufs=2) as outpool, \
             tc.tile_pool(name="ps", bufs=8, space="PSUM") as ps, \
             tc.tile_pool(name="dram", bufs=1, space="DRAM") as dpool:

            # ---- constants (diagonal A blocks + pb rows + scales) ----
            a0_t = cpool.tile([128, 128], dt.float32r, tag="a0")
            nc.sync.dma_start(out=a0_t[:, :], in_=amat_d.ap()[0:128, 0:128])
            a1_t = cpool.tile([128, 128], dt.float32r, tag="a1")
            nc.sync.dma_start(out=a1_t[:, :],
                              in_=amat_d.ap()[128:256, 128:256])
            apb_t = cpool.tile([PB, ROWS_D], dt.float32r, tag="apb")
            nc.sync.dma_start(out=apb_t[:, :], in_=amat_d.ap()[ROWS_D:AROWS, :])
            sc_t = cpool.tile([128, 2], dt.float32, tag="sc")
            nc.sync.dma_start(out=sc_t[:, 0:1], in_=scale_d.ap()[0:128])
            nc.sync.dma_start(out=sc_t[:, 1:2], in_=scale_d.ap()[128:256])
            w_t = cpool.tile([C, 3, 3, 3, C], dt.float32r, tag="w")
            nc.sync.dma_start(out=w_t[:, :, :, :, :], in_=wt_d.ap())
            b_t = cpool.tile([C, 3], dt.float32, tag="b")
            nc.sync.dma_start(out=b_t[:, :], in_=bt_d.ap())

            # ---- prefetch the first stripes' feats while the conv runs ----
            PREF = 5
            ktiles = {}
            for s in range(PREF):
                fs = slice(s * STRIPE, (s + 1) * STRIPE)
                k0 = inpool.tile([128, STRIPE], dt.float32r, tag="k0")
                nc.sync.dma_start(out=k0[:, :], in_=feats_d.ap()[0:128, fs])
                k1 = inpool.tile([128, STRIPE], dt.float32r, tag="k1")
                nc.sync.dma_start(out=k1[:, :], in_=feats_d.ap()[128:256, fs])
                ktiles[s] = (k0, k1)

            # ---- conv hypernet on this core's addr window ----
            pb_dram = dpool.tile([PB, F], dt.float32r, tag="pbd")
            cur = vpool.tile([C, PB, H, W], dt.float32r, tag="cur")
            nxt = vpool.tile([C, PB, H, W], dt.float32r, tag="nxt")
            pad = vpool.tile([C, PB, 18, 18], dt.float32r, tag="pad")
            nc.gpsimd.dma_start(out=pad[:, :, :, :], in_=zer_d.ap())
            nc.gpsimd.dma_start(
                out=cur[:, :, :, :].rearrange("c a h w -> c a (h w)"),
                in_=biasg_d.ap().rearrange("a c f -> c a f"))
            for l in range(3):
                lsrc, ldst = (cur, nxt) if l % 2 == 0 else (nxt, cur)
                # interior refresh; borders stay zero from the one-time init
                nc.vector.tensor_copy(pad[:, :, 1:17, 1:17],
                                      lsrc[:, :, :, :])
                for p in range(NPAIR):
                    pcv = ps.tile([128, 2, H, W], dt.float32, tag="ps")
                    for si in range(9):
                        ky, kx = divmod(si, 3)
                        nc.tensor.matmul(
                            pcv[:, :, :, :],
                            w_t[:, l, ky, kx, :],
                            pad[:, 2 * p:2 * p + 2, ky:ky + H, kx:kx + W],
                            start=(si == 0), stop=(si == 8))
                    nc.scalar.activation(
                        ldst[:, 2 * p:2 * p + 2, :, :], pcv[:, :, :, :],
                        mybir.ActivationFunctionType.Relu,
                        bias=b_t[:, l:l + 1], scale=1.0)
            res = nxt  # after 3 layers
            nc.gpsimd.dma_start(
                out=pb_dram[:, :].rearrange("a (c f) -> c a f", c=C),
                in_=res[:, :, :, :].rearrange("c a h w -> c a (h w)"))

            # ---- streamed segment-mean + pb matmul over feature stripes ----
            for s in range(NSTRIPES):
                fs = slice(s * STRIPE, (s + 1) * STRIPE)
                if s < PREF:
                    k0, k1 = ktiles.pop(s)
                else:
                    k0 = inpool.tile([128, STRIPE], dt.float32r, tag="k0")
                    nc.sync.dma_start(out=k0[:, :],
                                      in_=feats_d.ap()[0:128, fs])
                    k1 = inpool.tile([128, STRIPE], dt.float32r, tag="k1")
                    nc.sync.dma_start(out=k1[:, :],
                                      in_=feats_d.ap()[128:256, fs])
                pbs = pbpool.tile([PB, STRIPE], dt.float32r, tag="pbs")
                nc.gpsimd.dma_start(out=pbs[:, :], in_=pb_dram[:, fs])
                o0 = outpool.tile([128, STRIPE], dt.float32, tag="o0")
                o1 = outpool.tile([128, STRIPE], dt.float32, tag="o1")
                for t in range(NT):
                    ts = slice(t * 512, (t + 1) * 512)
                    p0 = ps.tile([128, 512], dt.float32, tag="ps")
                    nc.tensor.matmul(p0[:, :], a0_t[:, :], k0[:, ts],
                                     start=True, stop=False)
                    nc.tensor.matmul(p0[:, :], apb_t[:, 0:128], pbs[:, ts],
                                     start=False, stop=True)
                    p1 = ps.tile([128, 512], dt.float32, tag="ps")
                    nc.tensor.matmul(p1[:, :], a1_t[:, :], k1[:, ts],
                                     start=True, stop=False)
                    nc.tensor.matmul(p1[:, :], apb_t[:, 128:256], pbs[:, ts],
                                     start=False, stop=True)
                    nc.vector.tensor_scalar_mul(o0[:, ts], p0[:, :],
                                                sc_t[:, 0:1])
                    nc.scalar.mul(o1[:, ts], p1[:, :], sc_t[:, 1:2])
                nc.scalar.dma_start(out=out_d.ap()[0:128, fs], in_=o0[:, :])
                nc.scalar.dma_start(out=out_d.ap()[128:256, fs], in_=o1[:, :])

    nc.compile()
    _PROGRAM_CACHE["p"] = nc
    return nc


def _run(inputs, trace=False, trace_cores=None):
    style_ids = np.asarray(inputs["style_ids"])
    comp_ids = np.asarray(inputs["comp_ids"])
    comp_feats = np.ascontiguousarray(
        np.asarray(inputs["comp_feats"], dtype=np.float32))
    bias = np.asarray(inputs["bias"], dtype=np.float32).reshape(N_COMPS, C, HW)
    ws = [np.asarray(inputs[k], dtype=np.float32) for k in ("w1", "w2", "w3")]
    bs = [np.asarray(inputs[k], dtype=np.float32) for k in ("b1", "b2", "b3")]

    packed = _pack(style_ids, comp_ids)
    feats_flat = comp_feats.reshape(N_ITEMS, F)

    # [Cin, layer, ky, kx, Cout] from three [Cout, Cin, ky, kx]
    wt = np.transpose(np.stack(ws, axis=0), (2, 0, 3, 4, 1))
    wt = np.ascontiguousarray(wt, dtype=np.float32)
    bt = np.ascontiguousarray(np.stack(bs, axis=0).T, dtype=np.float32)

    in_maps = []
    for pk in packed:
        feats_c = np.zeros((ROWS_D, F), dtype=np.float32)
        feats_c[pk["used"]] = feats_flat[pk["rows"][pk["used"]]]
        biasg = np.zeros((PB, C, HW), dtype=np.float32)
        nsl = min(PB, N_COMPS - pk["addr_lo"])
        biasg[:nsl] = bias[pk["addr_lo"]:pk["addr_lo"] + nsl]
        in_maps.append({
            "feats": feats_c,
            "amat": pk["amat"],
            "scale": pk["scale"],
            "biasg": biasg,
            "wt": wt,
            "bt": bt,
            "zer": np.zeros((C, PB, 18, 18), dtype=np.float32),
        })

    nc = _build_program()
    res = bass_utils.run_bass_kernel_spmd(
        nc, in_maps, core_ids=list(range(N_CORES)), trace=trace,
        trace_cores=trace_cores)

    out_flat = np.zeros((N_ITEMS, F), dtype=np.float32)
    for pk, om in zip(packed, res.results):
        oc = om["outy"]
        out_flat[pk["rows"][pk["used"]]] = oc[pk["used"]]
    out = out_flat.reshape(B, 3, C, H, W)
    return out, res


def kernel(**inputs):
    out, _ = _run(inputs, trace=False)
    return out
